# revision 1
# baseline (speedup 1.0000x reference)
"""Betti-matching surrogate loss kernel for Trainium2 (8 NeuronCores).

Computes mean((probs - one_hot(gt_mask))^2) where gt_mask values are
{0,1,2} with ignore_index 2 mapped to class 0 (so class = (gt_mask == 1)).

Per chunk with t = (m == 1): DVE computes t (is_equal) and d_c = p_c ± t
(bf16), ACT squares and accumulates per-partition partial sums:
err0^2 = (1-(p0+t))^2, err1^2 = (p1-t)^2.

Sharding: core k = (b, g) with b = k // 4, g = k % 4 owns
probs[b, :, 8g:8g+8, :, :] and gt_mask[b, 8g:8g+8, :, :] — contiguous
zero-copy views of the full inputs. Each core computes per-partition
partial sums; the host reduces in float64.
"""

import os

import numpy as np

import concourse.bass as bass
import concourse.mybir as mybir
from concourse.bass_utils import run_bass_kernel_spmd
from concourse.tile import TileContext


import bass_rust


def split_multiwait_instructions(nc):
    """The walrus build in this image rejects any instruction carrying more
    than one sync wait ("Too many sync wait commands"). Tile's semaphore
    assignment freely attaches several. Hoist all but the last wait of each
    instruction onto injected same-engine NoOps placed directly before it —
    engine streams execute in order, so the waits still all complete before
    the real instruction issues."""
    k = 0
    for f in nc.m.functions:
        for bb in f.blocks:
            insts = bb.instructions
            out, changed = [], False
            for inst in insts:
                si = inst.sync_info
                if si is not None and si.on_wait and len(si.on_wait) > 1:
                    SI = type(si)
                    waits = list(si.on_wait)
                    for w in waits[:-1]:
                        nop = bass_rust.InstNoOp(
                            name=f"waitsplit-{k}",
                            engine=inst.engine,
                            sync_info=SI(on_wait=[w], on_update=[]),
                        )
                        k += 1
                        nc.register_instruction(nop)
                        out.append(nop)
                    inst.sync_info = SI(
                        on_wait=[waits[-1]], on_update=list(si.on_update)
                    )
                    changed = True
                out.append(inst)
            if changed:
                bb.instructions = out

def hoist_leading_dmas(nc, max_hoist=12):
    """Launch the input stream during the framework preamble: move the
    leading wait-free DMACopy instructions (any queue) out of the body
    block and into the entry block, ahead of the init-barrier Drain.
    The sequencers dispatch them asynchronously before joining the
    barrier, so the transfers overlap the const-memset/barrier preamble.
    Capped so the issuing engines don't delay the init barrier too long."""
    f = nc.m.functions[0]
    blocks = {bb.name: bb for bb in f.blocks}
    body = next(
        (bb for bb in f.blocks if "tile_context" in bb.name
         and not bb.name.endswith("_end")),
        None,
    )
    main = blocks.get("main")
    if body is None or main is None:
        return
    hoist = []
    engines = set()
    for inst in body.instructions:
        tn = type(inst).__name__
        if tn == "InstDMACopy":
            engines.add(inst.engine)
            if inst.sync_info is not None and inst.sync_info.on_wait:
                break
            hoist.append(inst)
            if len(hoist) >= max_hoist:
                break
        elif inst.engine in engines and (
            inst.sync_info is not None and inst.sync_info.on_wait
        ):
            break
    if not hoist:
        return
    names = {i.name for i in hoist}
    body.instructions = [i for i in body.instructions if i.name not in names]
    mi = main.instructions
    # Insert right after the entry InstCall: the SP sequencer then issues
    # the DMAs before its register moves, pulling the stream start forward.
    cut = 1 if mi and type(mi[0]).__name__ == "InstCall" else 0
    main.instructions = mi[:cut] + hoist + mi[cut:]


def overlap_final_store(nc):
    """Take the output-store DMAs' HBM-write receipt off the critical path.
    The kernel tail currently serializes: last ACT -> store DMA issue ->
    ~1.4us sem-update receipt -> end-block waits -> barriers -> epilogue.
    Nothing in the program consumes the stores' data or slots, and the
    wrapper epilogue (~7us of sem resets + cross-core barrier) runs after
    the end block, so the transfers complete long before the NEFF exits.
    Strip the stores' semaphore updates (so the epilogue's sem-file reset
    cannot race a late increment) and cap every wait on those lanes to the
    count still reachable from the remaining increments."""
    f = nc.m.functions[0]
    body = next(
        (bb for bb in f.blocks if "tile_context" in bb.name
         and not bb.name.endswith("_end")),
        None,
    )
    if body is None:
        return
    import bass_rust as br

    # The two accumulator-store DMAs are emitted last in the body block.
    stores = [
        i for i in body.instructions if type(i).__name__ == "InstDMACopy"
    ][-2:]
    stripped = {}
    for inst in stores:
        si = inst.sync_info
        if si is not None and si.on_update:
            zeroed = []
            for u in si.on_update:
                stripped[u.id] = stripped.get(u.id, 0) + (u.update_value or 0)
                zeroed.append(
                    br.SyncUpdate(
                        sync_type=u.sync_type,
                        id=u.id,
                        ant_name=u.ant_name,
                        update_mode=u.update_mode,
                        update_value=0,
                        update_reg=u.update_reg,
                    )
                )
            inst.sync_info = type(si)(
                on_wait=list(si.on_wait), on_update=zeroed
            )
    if not stripped:
        return
    # Final reachable count per sem = old final - stripped (the zeroed
    # updates no longer contribute). Tile's waits use absolute sem-ge-imm
    # values, so cap any wait above the new final.
    finals = {}
    for bb in f.blocks:
        for inst in bb.instructions:
            si = inst.sync_info
            if si is None:
                continue
            for u in si.on_update or []:
                if u.id in stripped:
                    finals[u.id] = finals.get(u.id, 0) + (u.update_value or 0)
    import bass_rust as br

    for bb in f.blocks:
        for inst in bb.instructions:
            si = inst.sync_info
            if si is None or not si.on_wait:
                continue
            if not any(
                w.id in stripped
                and w.wait_value is not None
                and w.wait_value > finals.get(w.id, 0)
                for w in si.on_wait
            ):
                continue
            new_waits = []
            for w in si.on_wait:
                if (
                    w.id in stripped
                    and w.wait_value is not None
                    and w.wait_value > finals.get(w.id, 0)
                ):
                    new_waits.append(
                        br.SyncWait(
                            sync_type=w.sync_type,
                            id=w.id,
                            ant_name=w.ant_name,
                            wait_mode=w.wait_mode,
                            wait_value=finals.get(w.id, 0),
                            wait_reg=w.wait_reg,
                        )
                    )
                else:
                    new_waits.append(w)
            inst.sync_info = type(si)(
                on_wait=new_waits, on_update=list(si.on_update)
            )


N_CORES = 8
B, C, D, H, W = 2, 2, 32, 512, 512
GROUPS = N_CORES // B          # 4 z-groups per batch
DG = D // GROUPS               # 8 z-slices per core
P = 128                        # SBUF partitions
TOTAL_W = DG * H * W // P      # 16384 free-dim elements per partition

# Per-partition chunk widths. Uniform 2048 keeps slot sizes small so the
# pools hold many chunks in flight (the DMA queue never starves); the
# tapered tail keeps the post-last-DMA compute drain short.
WIDTHS = [2048] * 7 + [1024, 768, 256]
assert sum(WIDTHS) == TOTAL_W

_nc_cache = {}
last_results = None


def build_nc(widths=WIDTHS, strip_second_barrier=False):
    """Per-core SPMD program: partial sums for one shard.

    strip_second_barrier drops the second end-block barrier round for a
    measured ~0.3us win, but a device-fatal NRT_EXEC_UNIT_UNRECOVERABLE
    appeared once during its test window (cause not isolated) — default
    off: the reliability risk is not worth 0.4%."""
    f32, i32 = mybir.dt.float32, mybir.dt.int32
    bf16, i8 = mybir.dt.bfloat16, mybir.dt.int8
    alu = mybir.AluOpType
    act = mybir.ActivationFunctionType

    total = TOTAL_W * P
    chunks, pos = [], 0
    for w in widths:
        chunks.append((pos, w))
        pos += P * w
    assert pos == total
    nch = len(chunks)
    ncols = 2 * nch

    nc = bass.Bass(enable_partition_id=False)
    p0 = nc.dram_tensor("p0", [total], f32, kind="ExternalInput")
    p1 = nc.dram_tensor("p1", [total], f32, kind="ExternalInput")
    m = nc.dram_tensor("m", [total], i32, kind="ExternalInput")
    out = nc.dram_tensor("out", [P, ncols], f32, kind="ExternalOutput")

    def chunk_ap(t, start, w):
        return t[start : start + P * w].rearrange("(p w) -> p w", p=P)

    with TileContext(nc) as tc:
        with (
            tc.tile_pool(name="acc", bufs=1) as acc_pool,
            tc.tile_pool(name="mp", bufs=4) as m_pool,
            tc.tile_pool(name="tp", bufs=3) as t_pool,
            tc.tile_pool(name="pp", bufs=8) as p_pool,
            tc.tile_pool(name="dp", bufs=4) as d_pool,
            tc.tile_pool(name="sq", bufs=3) as sq_pool,
        ):
            acc = acc_pool.tile([P, ncols], f32)
            for k, (start, w) in enumerate(chunks):
                mt = m_pool.tile([P, w], i32, tag="mt")
                nc.sync.dma_start(mt[:], chunk_ap(m, start, w))
                pt0 = p_pool.tile([P, w], f32, tag="pt")
                nc.sync.dma_start(pt0[:], chunk_ap(p0, start, w))
                pt1 = p_pool.tile([P, w], f32, tag="pt")
                nc.sync.dma_start(pt1[:], chunk_ap(p1, start, w))
                # t = 1.0 where mask == 1 (class 1), else 0.0
                tt = t_pool.tile([P, w], bf16, tag="tt")
                nc.vector.tensor_scalar(tt[:], mt[:], 1, None, op0=alu.is_equal)
                for ci, pt in ((0, pt0), (1, pt1)):
                    # c=0: d = p0 + t, err0^2 = Square(1 - d)
                    # c=1: d = p1 - t, err1^2 = Square(d)
                    op = alu.add if ci == 0 else alu.subtract
                    d = d_pool.tile([P, w], bf16, tag="d")
                    nc.vector.tensor_tensor(d[:], pt[:], tt[:], op=op)
                    sq = sq_pool.tile([P, w], bf16, tag="sq")
                    nc.scalar.activation(
                        sq[:], d[:], act.Square,
                        bias=1.0 if ci == 0 else 0.0,
                        scale=-1.0 if ci == 0 else 1.0,
                        accum_out=acc[:, 2 * k + ci : 2 * k + ci + 1],
                    )
            # Ship finished accumulator columns while the small tail
            # chunks still compute; issued after every input DMA, so the
            # input stream is never blocked behind this wait.
            split = 2 * (nch - 2)
            nc.sync.dma_start(out[:, :split], acc[:, :split])
            nc.sync.dma_start(out[:, split:], acc[:, split:])
    split_multiwait_instructions(nc)
    hoist_leading_dmas(nc)
    overlap_final_store(nc)
    if strip_second_barrier:
        # Drop the per-engine Drain+EventSemaphore round that follows the
        # event-range clear in the Tile end block. It only guarantees the
        # 51ns clear is visible before program end; the wrapper epilogue's
        # ~7us of sem resets runs after it regardless, and the barrier
        # bookkeeping sems are inside the range that epilogue resets.
        end = next(
            bb for bb in nc.m.functions[0].blocks if bb.name.endswith("_end")
        )
        insts = end.instructions
        isa_idx = next(
            (i for i, inst in enumerate(insts)
             if type(inst).__name__ == "InstISA"),
            None,
        )
        if isa_idx is not None:
            end.instructions = insts[: isa_idx + 1]
    nc.finalize()
    return nc


def _get_nc():
    if "nc" not in _nc_cache:
        _nc_cache["nc"] = build_nc()
    return _nc_cache["nc"]


def shard_inputs(probs, gt_mask):
    in_maps = []
    for k in range(N_CORES):
        b, g = divmod(k, GROUPS)
        z0 = g * DG
        in_maps.append(
            {
                "p0": probs[b, 0, z0 : z0 + DG].reshape(-1),
                "p1": probs[b, 1, z0 : z0 + DG].reshape(-1),
                "m": gt_mask[b, z0 : z0 + DG].reshape(-1),
            }
        )
    return in_maps


def kernel(probs, gt_mask):
    global last_results
    probs = np.ascontiguousarray(probs, dtype=np.float32)
    gt_mask = np.ascontiguousarray(gt_mask, dtype=np.int32)
    assert probs.shape == (B, C, D, H, W) and gt_mask.shape == (B, D, H, W)

    nc = _get_nc()
    in_maps = shard_inputs(probs, gt_mask)
    trace = bool(os.environ.get("BETTI_TRACE"))
    last_results = run_bass_kernel_spmd(
        nc, in_maps, core_ids=list(range(N_CORES)), trace=trace
    )
    total = 0.0
    for r in last_results.results:
        total += r["out"].astype(np.float64).sum()
    return np.asarray(total / (B * C * D * H * W), dtype=np.float32)



# revision 8
# speedup vs baseline: 1.0067x; 1.0067x over previous
"""Betti-matching surrogate loss kernel for Trainium2 (8 NeuronCores).

Computes mean((probs - one_hot(gt_mask))^2) where gt_mask values are
{0,1,2} with ignore_index 2 mapped to class 0 (so class = (gt_mask == 1)).

Identity used (u := (m-1)^2 in {0,1}, u = 1 - t where t = (m==1)):

    loss * N = sum(p0^2) + sum((p1-1)^2) + 2*sum(u * (p1 - p0))

which lets ACT square the prob planes directly (no dependency on the
mask path) while DVE computes the small correction term.

HBM traffic is the roofline for this problem, so the host shards AND
narrows dtypes while slicing: probs f32 -> bf16 (the device compute is
bf16 anyway; the loss shifts by ~1e-6 relative), gt_mask int32 -> int8
(lossless, values in {0,1,2}). Per-core bytes drop 24 MiB -> 10 MiB.

Per chunk: ACT computes u = Square(m-1) and Square(p1-1) with
per-partition accumulation; DVE computes sum(p0^2) via tensor_scalar
pow-accum (4x mode), q = p1-p0 and uq = u*q (2x mode), and sum(uq)
via tensor_scalar accum (4x). Host reduces the [P, 3*nch] partials
in float64.

Sharding: core k = (b, g) with b = k // 4, g = k % 4 owns
probs[b, :, 8g:8g+8, :, :] and gt_mask[b, 8g:8g+8, :, :] — contiguous
views of the dtype-narrowed full inputs.
"""

import os

import numpy as np

import concourse.bass as bass
import concourse.mybir as mybir
from concourse.bass_utils import run_bass_kernel_spmd
from concourse.tile import TileContext


import bass_rust


def split_multiwait_instructions(nc):
    """The walrus build in this image rejects any instruction carrying more
    than one sync wait ("Too many sync wait commands"). Tile's semaphore
    assignment freely attaches several. Hoist all but the last wait of each
    instruction onto injected same-engine NoOps placed directly before it —
    engine streams execute in order, so the waits still all complete before
    the real instruction issues."""
    k = 0
    for f in nc.m.functions:
        for bb in f.blocks:
            insts = bb.instructions
            out, changed = [], False
            for inst in insts:
                si = inst.sync_info
                if si is not None and si.on_wait and len(si.on_wait) > 1:
                    SI = type(si)
                    waits = list(si.on_wait)
                    for w in waits[:-1]:
                        nop = bass_rust.InstNoOp(
                            name=f"waitsplit-{k}",
                            engine=inst.engine,
                            sync_info=SI(on_wait=[w], on_update=[]),
                        )
                        k += 1
                        nc.register_instruction(nop)
                        out.append(nop)
                    inst.sync_info = SI(
                        on_wait=[waits[-1]], on_update=list(si.on_update)
                    )
                    changed = True
                out.append(inst)
            if changed:
                bb.instructions = out

def hoist_leading_dmas(nc, max_hoist=12):
    """Launch the input stream during the framework preamble: move the
    leading wait-free DMACopy instructions (any queue) out of the body
    block and into the entry block, ahead of the init-barrier Drain.
    The sequencers dispatch them asynchronously before joining the
    barrier, so the transfers overlap the const-memset/barrier preamble.
    Capped so the issuing engines don't delay the init barrier too long."""
    f = nc.m.functions[0]
    blocks = {bb.name: bb for bb in f.blocks}
    body = next(
        (bb for bb in f.blocks if "tile_context" in bb.name
         and not bb.name.endswith("_end")),
        None,
    )
    main = blocks.get("main")
    if body is None or main is None:
        return
    hoist = []
    engines = set()
    for inst in body.instructions:
        tn = type(inst).__name__
        if tn == "InstDMACopy":
            engines.add(inst.engine)
            if inst.sync_info is not None and inst.sync_info.on_wait:
                break
            hoist.append(inst)
            if len(hoist) >= max_hoist:
                break
        elif inst.engine in engines and (
            inst.sync_info is not None and inst.sync_info.on_wait
        ):
            break
    if not hoist:
        return
    names = {i.name for i in hoist}
    body.instructions = [i for i in body.instructions if i.name not in names]
    mi = main.instructions
    # Insert right after the entry InstCall: the SP sequencer then issues
    # the DMAs before its register moves, pulling the stream start forward.
    cut = 1 if mi and type(mi[0]).__name__ == "InstCall" else 0
    main.instructions = mi[:cut] + hoist + mi[cut:]


def overlap_final_store(nc, n_stores=2):
    """Take the output-store DMAs' HBM-write receipt off the critical path.
    The kernel tail otherwise serializes: last compute -> store DMA issue ->
    ~1.4us sem-update receipt -> end-block waits -> barriers -> epilogue.
    Nothing in the program consumes the stores' data or slots, and the
    wrapper epilogue (~7us of sem resets + cross-core barrier) runs after
    the end block, so the transfers complete long before the NEFF exits.
    Strip the stores' semaphore updates (so the epilogue's sem-file reset
    cannot race a late increment) and cap every wait on those lanes to the
    count still reachable from the remaining increments."""
    f = nc.m.functions[0]
    body = next(
        (bb for bb in f.blocks if "tile_context" in bb.name
         and not bb.name.endswith("_end")),
        None,
    )
    if body is None:
        return
    import bass_rust as br

    # The accumulator-store DMAs are emitted last in the body block.
    stores = [
        i for i in body.instructions if type(i).__name__ == "InstDMACopy"
    ][-n_stores:]
    stripped = {}
    for inst in stores:
        si = inst.sync_info
        if si is not None and si.on_update:
            zeroed = []
            for u in si.on_update:
                stripped[u.id] = stripped.get(u.id, 0) + (u.update_value or 0)
                zeroed.append(
                    br.SyncUpdate(
                        sync_type=u.sync_type,
                        id=u.id,
                        ant_name=u.ant_name,
                        update_mode=u.update_mode,
                        update_value=0,
                        update_reg=u.update_reg,
                    )
                )
            inst.sync_info = type(si)(
                on_wait=list(si.on_wait), on_update=zeroed
            )
    if not stripped:
        return
    # Final reachable count per sem = old final - stripped (the zeroed
    # updates no longer contribute). Tile's waits use absolute sem-ge-imm
    # values, so cap any wait above the new final.
    finals = {}
    for bb in f.blocks:
        for inst in bb.instructions:
            si = inst.sync_info
            if si is None:
                continue
            for u in si.on_update or []:
                if u.id in stripped:
                    finals[u.id] = finals.get(u.id, 0) + (u.update_value or 0)

    for bb in f.blocks:
        for inst in bb.instructions:
            si = inst.sync_info
            if si is None or not si.on_wait:
                continue
            if not any(
                w.id in stripped
                and w.wait_value is not None
                and w.wait_value > finals.get(w.id, 0)
                for w in si.on_wait
            ):
                continue
            new_waits = []
            for w in si.on_wait:
                if (
                    w.id in stripped
                    and w.wait_value is not None
                    and w.wait_value > finals.get(w.id, 0)
                ):
                    new_waits.append(
                        br.SyncWait(
                            sync_type=w.sync_type,
                            id=w.id,
                            ant_name=w.ant_name,
                            wait_mode=w.wait_mode,
                            wait_value=finals.get(w.id, 0),
                            wait_reg=w.wait_reg,
                        )
                    )
                else:
                    new_waits.append(w)
            inst.sync_info = type(si)(
                on_wait=new_waits, on_update=list(si.on_update)
            )


N_CORES = 8
B, C, D, H, W = 2, 2, 32, 512, 512
GROUPS = N_CORES // B          # 4 z-groups per batch
DG = D // GROUPS               # 8 z-slices per core
P = 128                        # SBUF partitions
TOTAL_W = DG * H * W // P      # 16384 free-dim elements per partition

# Per-partition chunk widths. Bigger leading chunks cut per-chunk ACT
# accumulator-read overhead; the tapered tail keeps the post-last-DMA
# compute drain short.
WIDTHS = [4096, 4096, 2048, 2048, 2048, 1024, 768, 256]
assert sum(WIDTHS) == TOTAL_W

_nc_cache = {}
last_results = None


def build_nc(widths=WIDTHS):
    """Per-core SPMD program: partial sums for one shard."""
    f32, i8 = mybir.dt.float32, mybir.dt.int8
    bf16 = mybir.dt.bfloat16
    alu = mybir.AluOpType
    act = mybir.ActivationFunctionType

    total = TOTAL_W * P
    chunks, pos = [], 0
    for w in widths:
        chunks.append((pos, w))
        pos += P * w
    assert pos == total
    nch = len(chunks)
    ncols = 3 * nch
    # ch0-square chunks assigned to ACT (~1/4 of the plane, taken from
    # the front so ACT races ahead while DVE fills its deeper pipeline).
    act_sq0_chunks = {0}

    nc = bass.Bass(enable_partition_id=False)
    p0 = nc.dram_tensor("p0", [total], bf16, kind="ExternalInput")
    p1 = nc.dram_tensor("p1", [total], bf16, kind="ExternalInput")
    m = nc.dram_tensor("m", [total], i8, kind="ExternalInput")
    out = nc.dram_tensor("out", [P, ncols], f32, kind="ExternalOutput")

    def chunk_ap(t, start, w):
        return t[start : start + P * w].rearrange("(p w) -> p w", p=P)

    with TileContext(nc) as tc:
        with (
            tc.tile_pool(name="acc", bufs=1) as acc_pool,
            tc.tile_pool(name="mp", bufs=4) as m_pool,
            tc.tile_pool(name="pp", bufs=6) as p_pool,
            tc.tile_pool(name="up", bufs=3) as u_pool,
            tc.tile_pool(name="qp", bufs=3) as q_pool,
            tc.tile_pool(name="uqp", bufs=3) as uq_pool,
            tc.tile_pool(name="so", bufs=2) as s_act_pool,
            tc.tile_pool(name="sv", bufs=3) as s_dve_pool,
        ):
            acc = acc_pool.tile([P, ncols], f32)
            for k, (start, w) in enumerate(chunks):
                mt = m_pool.tile([P, w], i8, tag="mt")
                nc.sync.dma_start(mt[:], chunk_ap(m, start, w))
                pt0 = p_pool.tile([P, w], bf16, tag="pt")
                nc.sync.dma_start(pt0[:], chunk_ap(p0, start, w))
                pt1 = p_pool.tile([P, w], bf16, tag="pt")
                nc.sync.dma_start(pt1[:], chunk_ap(p1, start, w))
                # ACT: u = (1-m)^2 = 1 - (m==1); exact for m in {0,1,2}
                ut = u_pool.tile([P, w], bf16, tag="ut")
                nc.scalar.activation(ut[:], mt[:], act.Square,
                                     bias=1.0, scale=-1.0)
                # ACT: acc[3k+1] = sum((1-p1)^2) = sum((p1-1)^2)
                sq1 = s_act_pool.tile([P, w], bf16, tag="sq1")
                nc.scalar.activation(
                    sq1[:], pt1[:], act.Square, bias=1.0, scale=-1.0,
                    accum_out=acc[:, 3 * k + 1 : 3 * k + 2],
                )
                # acc[3k+0] = sum(p0^2). ACT and DVE split this square by
                # chunk to balance engine load: ACT carries ~1/4 of the
                # plane (it has one spare slot beside u and the ch1
                # square), DVE (tensor_tensor mult + tensor_scalar accum)
                # carries the rest.
                if k in act_sq0_chunks:
                    sq0 = s_act_pool.tile([P, w], bf16, tag="sq1")
                    nc.scalar.activation(
                        sq0[:], pt0[:], act.Square,
                        accum_out=acc[:, 3 * k : 3 * k + 1],
                    )
                else:
                    sq0 = s_dve_pool.tile([P, w], bf16, tag="sv")
                    nc.vector.tensor_tensor(
                        sq0[:], pt0[:], pt0[:], op=alu.mult
                    )
                    s0o = s_dve_pool.tile([P, w], bf16, tag="sv")
                    nc.vector.tensor_scalar(
                        s0o[:], sq0[:], 1.0, 0.0, op0=alu.mult, op1=alu.add,
                        accum_out=acc[:, 3 * k : 3 * k + 1],
                    )
                # DVE: q = p1 - p0 (2x), uq = u*q (2x)
                qt = q_pool.tile([P, w], bf16, tag="qt")
                nc.vector.tensor_tensor(qt[:], pt1[:], pt0[:], op=alu.subtract)
                uqt = uq_pool.tile([P, w], bf16, tag="uqt")
                nc.vector.tensor_tensor(uqt[:], ut[:], qt[:], op=alu.mult)
                # DVE: acc[3k+2] = sum(uq)   (4x-mode tensor_scalar)
                uqo = s_dve_pool.tile([P, w], bf16, tag="sv")
                nc.vector.tensor_scalar(
                    uqo[:], uqt[:], 1.0, 0.0, op0=alu.mult, op1=alu.add,
                    accum_out=acc[:, 3 * k + 2 : 3 * k + 3],
                )
            # Ship finished accumulator columns while the small tail
            # chunks still compute; issued after every input DMA, so the
            # input stream is never blocked behind this wait.
            split = 3 * (nch - 2)
            nc.sync.dma_start(out[:, :split], acc[:, :split])
            nc.sync.dma_start(out[:, split:], acc[:, split:])
    split_multiwait_instructions(nc)
    hoist_leading_dmas(nc)
    overlap_final_store(nc, n_stores=2)
    nc.finalize()
    return nc


def _get_nc():
    if "nc" not in _nc_cache:
        _nc_cache["nc"] = build_nc()
    return _nc_cache["nc"]


def shard_inputs(probs, gt_mask):
    import ml_dtypes

    pb = probs.astype(ml_dtypes.bfloat16)    # (B,C,D,H,W) bf16
    mb = gt_mask.astype(np.int8)             # (B,D,H,W) i8, values {0,1,2}
    in_maps = []
    for k in range(N_CORES):
        b, g = divmod(k, GROUPS)
        z0 = g * DG
        in_maps.append(
            {
                "p0": pb[b, 0, z0 : z0 + DG].reshape(-1),
                "p1": pb[b, 1, z0 : z0 + DG].reshape(-1),
                "m": mb[b, z0 : z0 + DG].reshape(-1),
            }
        )
    return in_maps


def kernel(probs, gt_mask):
    global last_results
    probs = np.ascontiguousarray(probs, dtype=np.float32)
    gt_mask = np.ascontiguousarray(gt_mask, dtype=np.int32)
    assert probs.shape == (B, C, D, H, W) and gt_mask.shape == (B, D, H, W)

    nc = _get_nc()
    in_maps = shard_inputs(probs, gt_mask)
    trace = bool(os.environ.get("BETTI_TRACE"))
    last_results = run_bass_kernel_spmd(
        nc, in_maps, core_ids=list(range(N_CORES)), trace=trace
    )
    total = 0.0
    for r in last_results.results:
        a = r["out"].astype(np.float64)       # [P, 3*nch]
        total += a[:, 0::3].sum() + a[:, 1::3].sum() + 2.0 * a[:, 2::3].sum()
    return np.asarray(total / (B * C * D * H * W), dtype=np.float32)


# revision 12
# speedup vs baseline: 1.5277x; 1.5175x over previous
"""Betti-matching surrogate loss kernel for Trainium2 (8 NeuronCores).

Computes mean((probs - one_hot(gt_mask))^2) where gt_mask values are
{0,1,2} with ignore_index 2 mapped to class 0 (so class = (gt_mask == 1)).

Identity used (u := (1-m)^2 in {0,1}, u = 1 - t where t = (m==1)):

    loss * N = sum(p0^2) + sum((p1-1)^2) + 2*sum(u * (p1 - p0))

HBM traffic is the roofline for this problem, so the host narrows
dtypes while sharding: probs f32 -> bf16 (device compute is bf16
anyway; the loss shifts ~1e-6 relative), gt_mask int32 -> int8
(lossless). Per-core bytes drop 24 MiB -> 10 MiB (~29 us at line rate).

Engine split, chosen from measured rates (ACT pass 13.7us/plane, DVE
tensor_tensor 2x 8.5us/plane, DVE tensor_scalar+accum only 1x, PE
ones-matmul reduce 7-14us/plane on an otherwise idle engine):

  ACT: u = Square(1-m) [i8 in], acc1 = Square(1-p1) with accumulate
  DVE: sq0 = p0*p0, q = p1-p0, uq = u*q          (three 2x passes)
  PE : sum(sq0), sum(uq) via ones-vector matmuls into PSUM
       (bulk group stored early; small tail group drains last chunks)

Sharding: core k = (b, g) with b = k // 4, g = k % 4 owns
probs[b, :, 8g:8g+8, :, :] and gt_mask[b, 8g:8g+8, :, :] — contiguous
views of the dtype-narrowed full inputs. Host reduces partials in f64.
"""

import os

import numpy as np

import concourse.bass as bass
import concourse.mybir as mybir
from concourse.bass_utils import run_bass_kernel_spmd
from concourse.tile import TileContext


import bass_rust


def split_multiwait_instructions(nc):
    """The walrus build in this image rejects any instruction carrying more
    than one sync wait ("Too many sync wait commands"). Tile's semaphore
    assignment freely attaches several. Hoist all but the last wait of each
    instruction onto injected same-engine NoOps placed directly before it —
    engine streams execute in order, so the waits still all complete before
    the real instruction issues."""
    k = 0
    for f in nc.m.functions:
        for bb in f.blocks:
            insts = bb.instructions
            out, changed = [], False
            for inst in insts:
                si = inst.sync_info
                if si is not None and si.on_wait and len(si.on_wait) > 1:
                    SI = type(si)
                    waits = list(si.on_wait)
                    for w in waits[:-1]:
                        nop = bass_rust.InstNoOp(
                            name=f"waitsplit-{k}",
                            engine=inst.engine,
                            sync_info=SI(on_wait=[w], on_update=[]),
                        )
                        k += 1
                        nc.register_instruction(nop)
                        out.append(nop)
                    inst.sync_info = SI(
                        on_wait=[waits[-1]], on_update=list(si.on_update)
                    )
                    changed = True
                out.append(inst)
            if changed:
                bb.instructions = out

def hoist_leading_dmas(nc, max_hoist=12):
    """Launch the input stream during the framework preamble: move the
    leading wait-free DMACopy instructions (any queue) out of the body
    block and into the entry block, ahead of the init-barrier Drain.
    The sequencers dispatch them asynchronously before joining the
    barrier, so the transfers overlap the const-memset/barrier preamble.
    Capped so the issuing engines don't delay the init barrier too long."""
    f = nc.m.functions[0]
    blocks = {bb.name: bb for bb in f.blocks}
    body = next(
        (bb for bb in f.blocks if "tile_context" in bb.name
         and not bb.name.endswith("_end")),
        None,
    )
    main = blocks.get("main")
    if body is None or main is None:
        return
    hoist = []
    engines = set()
    for inst in body.instructions:
        tn = type(inst).__name__
        if tn == "InstDMACopy":
            engines.add(inst.engine)
            if inst.sync_info is not None and inst.sync_info.on_wait:
                break
            hoist.append(inst)
            if len(hoist) >= max_hoist:
                break
        elif inst.engine in engines and (
            inst.sync_info is not None and inst.sync_info.on_wait
        ):
            break
    if not hoist:
        return
    names = {i.name for i in hoist}
    body.instructions = [i for i in body.instructions if i.name not in names]
    mi = main.instructions
    # Insert right after the entry InstCall: the SP sequencer then issues
    # the DMAs before its register moves, pulling the stream start forward.
    cut = 1 if mi and type(mi[0]).__name__ == "InstCall" else 0
    main.instructions = mi[:cut] + hoist + mi[cut:]


def overlap_final_store(nc, n_stores=2):
    """Take the output-store DMAs' HBM-write receipt off the critical path.
    The kernel tail otherwise serializes: last compute -> store DMA issue ->
    ~1.4us sem-update receipt -> end-block waits -> barriers -> epilogue.
    Nothing in the program consumes the stores' data or slots, and the
    wrapper epilogue (~7us of sem resets + cross-core barrier) runs after
    the end block, so the transfers complete long before the NEFF exits.
    Strip the stores' semaphore updates (so the epilogue's sem-file reset
    cannot race a late increment) and cap every wait on those lanes to the
    count still reachable from the remaining increments."""
    f = nc.m.functions[0]
    body = next(
        (bb for bb in f.blocks if "tile_context" in bb.name
         and not bb.name.endswith("_end")),
        None,
    )
    if body is None:
        return
    import bass_rust as br

    # The accumulator-store DMAs are emitted last in the body block.
    stores = [
        i for i in body.instructions if type(i).__name__ == "InstDMACopy"
    ][-n_stores:]
    stripped = {}
    for inst in stores:
        si = inst.sync_info
        if si is not None and si.on_update:
            zeroed = []
            for u in si.on_update:
                stripped[u.id] = stripped.get(u.id, 0) + (u.update_value or 0)
                zeroed.append(
                    br.SyncUpdate(
                        sync_type=u.sync_type,
                        id=u.id,
                        ant_name=u.ant_name,
                        update_mode=u.update_mode,
                        update_value=0,
                        update_reg=u.update_reg,
                    )
                )
            inst.sync_info = type(si)(
                on_wait=list(si.on_wait), on_update=zeroed
            )
    if not stripped:
        return
    # Final reachable count per sem = old final - stripped (the zeroed
    # updates no longer contribute). Tile's waits use absolute sem-ge-imm
    # values, so cap any wait above the new final.
    finals = {}
    for bb in f.blocks:
        for inst in bb.instructions:
            si = inst.sync_info
            if si is None:
                continue
            for u in si.on_update or []:
                if u.id in stripped:
                    finals[u.id] = finals.get(u.id, 0) + (u.update_value or 0)

    for bb in f.blocks:
        for inst in bb.instructions:
            si = inst.sync_info
            if si is None or not si.on_wait:
                continue
            if not any(
                w.id in stripped
                and w.wait_value is not None
                and w.wait_value > finals.get(w.id, 0)
                for w in si.on_wait
            ):
                continue
            new_waits = []
            for w in si.on_wait:
                if (
                    w.id in stripped
                    and w.wait_value is not None
                    and w.wait_value > finals.get(w.id, 0)
                ):
                    new_waits.append(
                        br.SyncWait(
                            sync_type=w.sync_type,
                            id=w.id,
                            ant_name=w.ant_name,
                            wait_mode=w.wait_mode,
                            wait_value=finals.get(w.id, 0),
                            wait_reg=w.wait_reg,
                        )
                    )
                else:
                    new_waits.append(w)
            inst.sync_info = type(si)(
                on_wait=new_waits, on_update=list(si.on_update)
            )


N_CORES = 8
B, C, D, H, W = 2, 2, 32, 512, 512
GROUPS = N_CORES // B          # 4 z-groups per batch
DG = D // GROUPS               # 8 z-slices per core
P = 128                        # SBUF partitions
TOTAL_W = DG * H * W // P      # 16384 free-dim elements per partition
PLANE = TOTAL_W * P            # elements per (core, channel) plane

# Per-partition chunk widths. Bigger leading chunks cut per-instruction
# and per-event overhead; the tapered tail keeps the post-last-DMA
# compute drain short. The last N_TAIL chunks form the separate PE
# accumulation group whose store happens at the very end.
WIDTHS = [4096, 4096, 4096, 2048, 1024, 1024]
N_TAIL = 2
assert sum(WIDTHS) == TOTAL_W

_nc_cache = {}
last_results = None


def build_nc(widths=WIDTHS, n_tail=N_TAIL):
    """Per-core SPMD program: partial sums for one shard."""
    f32, i8 = mybir.dt.float32, mybir.dt.int8
    bf16 = mybir.dt.bfloat16
    alu = mybir.AluOpType
    act = mybir.ActivationFunctionType

    chunks, pos = [], 0
    for w in widths:
        chunks.append((pos, w))
        pos += P * w
    assert pos == PLANE
    nch = len(chunks)
    n_bulk = nch - n_tail

    nc = bass.Bass(enable_partition_id=False)
    # p holds both channels: [0:PLANE] = p0, [PLANE:2*PLANE] = p1
    p = nc.dram_tensor("p", [2 * PLANE], bf16, kind="ExternalInput")
    m = nc.dram_tensor("m", [PLANE], i8, kind="ExternalInput")
    # out: acc1 columns (ACT ch1 accum, one col/chunk) stored per half,
    # out2: PE-reduced rows [4, 512] = (s0 bulk, uq bulk, s0 tail, uq tail)
    out = nc.dram_tensor("out", [P, nch], f32, kind="ExternalOutput")
    # out2 rows (flat): [0:512]=s0 bulk, [512:1024]=uq bulk,
    #                   [1024:1536]=s0 tail, [1536:2048]=uq tail
    out2 = nc.dram_tensor("out2", [2048], f32, kind="ExternalOutput")

    ones = nc.const_aps.aps[(bf16, 1.0)]     # [128, 1] SBUF constant

    def chunk_ap(t, base, start, w):
        return t[base + start : base + start + P * w].rearrange(
            "(p w) -> p w", p=P
        )

    with TileContext(nc) as tc:
        with (
            tc.tile_pool(name="acc", bufs=1) as acc_pool,
            tc.tile_pool(name="mp", bufs=3) as m_pool,
            tc.tile_pool(name="pp", bufs=3) as p_pool,
            tc.tile_pool(name="up", bufs=2) as u_pool,
            tc.tile_pool(name="qp", bufs=2) as q_pool,
            tc.tile_pool(name="uqp", bufs=2) as uq_pool,
            tc.tile_pool(name="s0p", bufs=2) as s0_pool,
            tc.tile_pool(name="sap", bufs=2) as s_act_pool,
            tc.tile_pool(name="fin", bufs=1) as fin_pool,
            tc.psum_pool(name="ps", bufs=1) as psum_pool,
        ):
            acc = acc_pool.tile([P, nch], f32)
            ps_s0_b = psum_pool.tile([1, 512], f32)
            ps_uq_b = psum_pool.tile([1, 512], f32)
            ps_s0_t = psum_pool.tile([1, 512], f32)
            ps_uq_t = psum_pool.tile([1, 512], f32)
            fin = fin_pool.tile([1, 1024], f32)

            started = {id(ps_s0_b): False, id(ps_uq_b): False,
                       id(ps_s0_t): False, id(ps_uq_t): False}
            # count matmuls per psum target so stop lands on the last one
            mm_total = {id(ps_s0_b): 0, id(ps_uq_b): 0,
                        id(ps_s0_t): 0, id(ps_uq_t): 0}
            for k, (start, w) in enumerate(chunks):
                n = (id(ps_s0_b) if k < n_bulk else id(ps_s0_t))
                mm_total[n] += (w + 511) // 512
            mm_total[id(ps_uq_b)] = mm_total[id(ps_s0_b)]
            mm_total[id(ps_uq_t)] = mm_total[id(ps_s0_t)]
            mm_done = {kk: 0 for kk in mm_total}

            def pe_reduce(src, w, ps):
                g0 = 0
                while g0 < w:
                    gw = min(512, w - g0)
                    mm_done[id(ps)] += 1
                    nc.tensor.matmul(
                        ps[:, :gw],
                        ones,
                        src[:, g0 : g0 + gw],
                        start=not started[id(ps)],
                        stop=mm_done[id(ps)] == mm_total[id(ps)],
                    )
                    started[id(ps)] = True
                    g0 += gw

            for k, (start, w) in enumerate(chunks):
                tail = k >= n_bulk
                mt = m_pool.tile([P, w], i8, tag="mt")
                nc.sync.dma_start(mt[:], chunk_ap(m, 0, start, w))
                # both prob channels in one DMA / one tile
                pt = p_pool.tile([P, 2 * w], bf16, tag="pt")
                nc.sync.dma_start(pt[:, :w], chunk_ap(p, 0, start, w))
                nc.sync.dma_start(pt[:, w:], chunk_ap(p, PLANE, start, w))
                pt0, pt1 = pt[:, :w], pt[:, w:]
                # ACT: u = (1-m)^2 = 1 - (m==1); exact for m in {0,1,2}
                ut = u_pool.tile([P, w], bf16, tag="ut")
                nc.scalar.activation(ut[:], mt[:], act.Square,
                                     bias=1.0, scale=-1.0)
                # ACT: acc[k] = sum((1-p1)^2)
                sq1 = s_act_pool.tile([P, w], bf16, tag="sq1")
                nc.scalar.activation(
                    sq1[:], pt1, act.Square, bias=1.0, scale=-1.0,
                    accum_out=acc[:, k : k + 1],
                )
                # DVE (all 2x): sq0 = p0*p0 ; q = p1-p0 ; uq = u*q
                sq0 = s0_pool.tile([P, w], bf16, tag="s0")
                nc.vector.tensor_tensor(sq0[:], pt0, pt0, op=alu.mult)
                qt = q_pool.tile([P, w], bf16, tag="qt")
                nc.vector.tensor_tensor(qt[:], pt1, pt0, op=alu.subtract)
                uqt = uq_pool.tile([P, w], bf16, tag="uqt")
                nc.vector.tensor_tensor(uqt[:], ut[:], qt[:], op=alu.mult)
                # PE: accumulate plane sums in PSUM
                pe_reduce(sq0, w, ps_s0_t if tail else ps_s0_b)
                pe_reduce(uqt, w, ps_uq_t if tail else ps_uq_b)
                if k == n_bulk - 1:
                    # bulk groups complete: drain them to SBUF and ship
                    # together with the bulk ACT columns while the tail
                    # chunks still compute.
                    nc.vector.tensor_copy(fin[:, 0:512], ps_s0_b[:, :])
                    nc.vector.tensor_copy(fin[:, 512:1024], ps_uq_b[:, :])
                    nc.sync.dma_start(
                        out2[0:1024].rearrange("(p w) -> p w", p=1), fin[:, :]
                    )
                    nc.sync.dma_start(out[:, :n_bulk], acc[:, :n_bulk])
            fin2 = fin_pool.tile([1, 1024], f32)
            nc.vector.tensor_copy(fin2[:, 0:512], ps_s0_t[:, :])
            nc.vector.tensor_copy(fin2[:, 512:1024], ps_uq_t[:, :])
            nc.sync.dma_start(
                out2[1024:2048].rearrange("(p w) -> p w", p=1), fin2[:, :]
            )
            nc.sync.dma_start(out[:, n_bulk:], acc[:, n_bulk:])
    split_multiwait_instructions(nc)
    hoist_leading_dmas(nc)
    overlap_final_store(nc, n_stores=2)
    nc.finalize()
    return nc


def _get_nc():
    if "nc" not in _nc_cache:
        _nc_cache["nc"] = build_nc()
    return _nc_cache["nc"]


def shard_inputs(probs, gt_mask):
    import ml_dtypes

    pb = probs.astype(ml_dtypes.bfloat16)    # (B,C,D,H,W) bf16
    mb = gt_mask.astype(np.int8)             # (B,D,H,W) i8, values {0,1,2}
    in_maps = []
    for k in range(N_CORES):
        b, g = divmod(k, GROUPS)
        z0 = g * DG
        in_maps.append(
            {
                # both channels contiguous: [p0 plane | p1 plane]
                "p": pb[b, :, z0 : z0 + DG].reshape(-1),
                "m": mb[b, z0 : z0 + DG].reshape(-1),
            }
        )
    return in_maps


def kernel(probs, gt_mask):
    global last_results
    probs = np.ascontiguousarray(probs, dtype=np.float32)
    gt_mask = np.ascontiguousarray(gt_mask, dtype=np.int32)
    assert probs.shape == (B, C, D, H, W) and gt_mask.shape == (B, D, H, W)

    nc = _get_nc()
    in_maps = shard_inputs(probs, gt_mask)
    trace = bool(os.environ.get("BETTI_TRACE"))
    last_results = run_bass_kernel_spmd(
        nc, in_maps, core_ids=list(range(N_CORES)), trace=trace
    )
    total = 0.0
    for r in last_results.results:
        a = r["out"].astype(np.float64)       # [P, nch]  sum((1-p1)^2) cols
        b = r["out2"].astype(np.float64).reshape(4, 512)  # s0b,uqb,s0t,uqt
        total += a.sum() + (b[0] + b[2]).sum() + 2.0 * (b[1] + b[3]).sum()
    return np.asarray(total / (B * C * D * H * W), dtype=np.float32)
